# revision 9
# baseline (speedup 1.0000x reference)
"""Sliding-window GQA causal self-attention on 8 TRN2 NeuronCores.

Sharding: sequence-parallel. B=2 batches x 4 chunks of 512 tokens = 8 shards.
Sliding window (512) makes attention local: each chunk only needs the 512
preceding tokens (halo) for K/V, so there are NO collectives. Each core:
  qkv proj (bf16 matmul, k-outer with 4 PSUM groups so the PE starts ~1.5us
  into the kernel and stays HAM-warm) -> rotate-half RoPE + RMS-norm (row
  layout) -> DMA x-bar block transposes into [dh, tok] layout (no PE
  transposes) -> banded attention (scores batched over query tiles per key
  tile, exp'd probs feed attn@V as lhsT, softmax denominators fused via a
  ones-column in V) -> output projection with streamed weight columns.
Host pre-transposes x / weights, de-interleaves RoPE pairs into the qkv
weight so RoPE is a contiguous rotate-half, and precomputes cos/sin tables
and the multiplicative band mask per core.
"""

import sys

sys.path.insert(0, "/opt/trn_rl_repo")

import numpy as np
import ml_dtypes

import concourse.bass as bass
import concourse.mybir as mybir
import concourse.tile as tile
from concourse import bacc
from concourse.bass_utils import run_bass_kernel_spmd


def _install_ntff_hook():
    """antenv.axon_hooks is absent on this image; bridge the ctypes NTFF
    profiling hook from trn_agent_boot so trace=True works."""
    import types
    try:
        import antenv.axon_hooks  # noqa: F401
        return
    except ImportError:
        pass
    try:
        from trn_agent_boot.trn_boot import _ntff_profile_via_ctypes
        hook = _ntff_profile_via_ctypes("/opt/axon/libaxon_pjrt.so")
    except Exception:
        hook = None
    mod = types.ModuleType("antenv.axon_hooks")
    mod.get_axon_ntff_profile_hook = lambda: hook
    mod.set_axon_ntff_profile_hook = lambda h: None
    sys.modules["antenv.axon_hooks"] = mod


_install_ntff_hook()

BF16 = mybir.dt.bfloat16
F32 = mybir.dt.float32

B, T, C = 2, 2048, 2048
HQ, HKV, DH, WIN = 16, 4, 128, 512
RMS_EPS = 1.1920929e-07
CHUNK = 512          # own tokens per core
WTOK = 1024          # window tokens per core (halo 512 + own 512)
NKT = WTOK // 128    # 8 k-tiles
NQT = CHUNK // 128   # 4 q-tiles
KWIN = 640           # keys per q-tile (5 blocks of 128)
SCALE = 1.0 / np.sqrt(DH)

_NC_CACHE = [None]


def _build_nc():
    nc = bacc.Bacc("TRN2", target_bir_lowering=False, debug=False, num_devices=8)

    xt = nc.declare_dram_parameter("xt", [C, WTOK], BF16, False)        # x^T window
    wq = nc.declare_dram_parameter("wq", [C, (HQ + 2 * HKV) * DH], BF16, False)
    wp = nc.declare_dram_parameter("wp", [C, C], BF16, False)
    csq = nc.declare_dram_parameter("csq", [CHUNK, HQ * DH], BF16, False)
    ssq = nc.declare_dram_parameter("ssq", [CHUNK, HQ * DH], BF16, False)
    csk = nc.declare_dram_parameter("csk", [WTOK, HKV * DH], BF16, False)
    ssk = nc.declare_dram_parameter("ssk", [WTOK, HKV * DH], BF16, False)
    mask = nc.declare_dram_parameter("mask", [NQT, 128, 256], BF16, False)
    vld = nc.declare_dram_parameter("vld", [128, NKT * HKV], BF16, False)
    out = nc.declare_dram_parameter("out", [CHUNK, C], F32, True)

    xt_r = xt.ap().rearrange("(k p) t -> k p t", p=128)       # 16 x [128,1024]
    wq_r = wq.ap().rearrange("(k p) m -> p k m", p=128)       # [128, 16, 3072]
    wp_r = wp.ap().rearrange("(k p) m -> p k m", p=128)       # [128, 16, 2048]
    csq_r = csq.ap().rearrange("(n p) d -> n p d", p=128)     # 4 x [128,2048]
    ssq_r = ssq.ap().rearrange("(n p) d -> n p d", p=128)
    csk_r = csk.ap().rearrange("(n p) d -> n p d", p=128)     # 8 x [128,512]
    ssk_r = ssk.ap().rearrange("(n p) d -> n p d", p=128)

    with tile.TileContext(nc) as tc:
        with (
            tc.tile_pool(name="singles", bufs=1) as singles,
            tc.tile_pool(name="wtiles", bufs=2) as wtiles,      # streamed weight cols
            tc.tile_pool(name="raw", bufs=1) as raw,            # Q/K raw + V
            tc.tile_pool(name="tbl", bufs=2) as tbl,            # rope tables (rotating)
            tc.tile_pool(name="work", bufs=2) as work,          # small DVE temps
            tc.tile_pool(name="pe", bufs=12) as pepool,         # exp'd probs
            tc.tile_pool(name="pp", bufs=4, space="PSUM") as pp_pool,   # proj (4 banks)
            tc.tile_pool(name="psA", bufs=2, space="PSUM") as psA,      # scores
            tc.tile_pool(name="psB", bufs=2, space="PSUM") as psB,      # AV out
        ):
            eps_t = singles.tile([128, 1], F32)
            nc.vector.memset(eps_t, RMS_EPS)

            # resident x^T: 16 tiles [128, 1024] (C-tile, tok); first column's
            # weights interleaved with the xt stream so the PE starts early
            xts = [singles.tile([128, WTOK], BF16, name=f"xts{k}") for k in range(16)]
            wc_k = wtiles.tile([128, 16, 512], BF16, tag="wcol", name="wck")
            for c4 in range(4):
                for k in range(4 * c4, 4 * c4 + 4):
                    nc.sync.dma_start(out=xts[k], in_=xt_r[k])
                nc.sync.dma_start(
                    out=wc_k[:, 4 * c4:4 * c4 + 4, :],
                    in_=wq_r[:, 4 * c4:4 * c4 + 4, 4 * 512:5 * 512])

            def load_wcol(src_r, n, name):
                wc = wtiles.tile([128, 16, 512], BF16, tag="wcol", name=name)
                for c4 in range(4):
                    nc.sync.dma_start(
                        out=wc[:, 4 * c4:4 * c4 + 4, :],
                        in_=src_r[:, 4 * c4:4 * c4 + 4, n * 512:(n + 1) * 512])
                return wc

            # raw (pre-rope) Q/K; V with ones column (softmax denominator)
            qraw = [raw.tile([128, HQ, DH], BF16, name=f"qraw{i}") for i in range(NQT)]
            kraw = [raw.tile([128, HKV, DH], BF16, name=f"kraw{j}") for j in range(NKT)]
            vext = raw.tile([128, NKT, HKV, 132], BF16, name="vext")
            nc.gpsimd.dma_start(out=vext[:, :, :, 128:129], in_=vld.ap())

            # ---------------- QKV projection (k-outer, 4 psum groups) ------
            # feature columns: n=0..3 Q (heads 4n..4n+3), n=4 K, n=5 V
            def proj_col(n, wc):
                itiles = list(range(4, 8)) if n < 4 else list(range(8))
                for half in range(len(itiles) // 4):
                    igrp = itiles[half * 4:half * 4 + 4]
                    pps = {i: pp_pool.tile([128, 512], F32, tag="pp", name=f"pp{n}_{i}")
                           for i in igrp}
                    for k in range(16):
                        for i in igrp:
                            nc.tensor.matmul(pps[i], xts[k][:, i * 128:(i + 1) * 128],
                                             wc[:, k, :],
                                             start=(k == 0), stop=(k == 15))
                    for i in igrp:
                        if n < 4:
                            qi = i - 4
                            nc.scalar.copy(
                                out=qraw[qi][:, 4 * n:4 * n + 4, :].rearrange("p a b -> p (a b)"),
                                in_=pps[i][:])
                        elif n == 4:
                            nc.scalar.copy(
                                out=kraw[i][:, :, :].rearrange("p a b -> p (a b)"),
                                in_=pps[i][:])
                        else:
                            nc.scalar.copy(
                                out=vext[:, i, :, 0:128],
                                in_=pps[i].rearrange("p (g d) -> p g d", g=HKV))

            # ---------------- RoPE + RMS-norm (row layout) -----------------
            qt = singles.tile([128, HQ, NQT, 128], BF16)   # [dh, h, qi, tok]
            kt = singles.tile([128, HKV, NKT, 128], BF16)  # [dh, g, j, tok]

            def rope_rms(src, h, sfx, cs_src, ss_src, dst_sl):
                # src: [128 tok, h, 128 dh] bf16 raw -> normalized rope'd rows,
                # then one DMA x-bar block transpose into dst_sl [dh, h, tok].
                cs = tbl.tile([128, h, DH], BF16, tag=f"cs{sfx}", name=f"cs{sfx}")
                ss = tbl.tile([128, h, DH], BF16, tag=f"ss{sfx}", name=f"ss{sfx}")
                nc.gpsimd.dma_start(out=cs.rearrange("p a b -> p (a b)"), in_=cs_src)
                nc.gpsimd.dma_start(out=ss.rearrange("p a b -> p (a b)"), in_=ss_src)

                tmp = work.tile([128, h, DH], BF16, tag=f"tmp{sfx}")
                ro = work.tile([128, h, DH], BF16, tag=f"ro{sfx}")
                # squared sums (rope is a rotation: norms commute), tmp as scratch
                nc.vector.tensor_mul(tmp, src, src)
                ssum = work.tile([128, h], F32, tag=f"ssum{sfx}")
                nc.vector.reduce_sum(out=ssum, in_=tmp, axis=mybir.AxisListType.X)
                rstd = work.tile([128, h], F32, tag=f"rstd{sfx}")
                nc.scalar.activation(rstd, ssum, mybir.ActivationFunctionType.Sqrt,
                                     bias=eps_t[:], scale=1.0 / DH)
                rinv = work.tile([128, h], F32, tag=f"rinv{sfx}")
                nc.vector.reciprocal(rinv, rstd)

                nc.vector.tensor_mul(tmp[:, :, 0:64], src[:, :, 64:128], ss[:, :, 0:64])
                nc.vector.tensor_mul(tmp[:, :, 64:128], src[:, :, 0:64], ss[:, :, 64:128])
                nc.vector.tensor_mul(ro, src, cs)
                nc.vector.tensor_add(ro, ro, tmp)
                for hh in range(h):
                    nc.vector.tensor_scalar_mul(ro[:, hh, :], ro[:, hh, :],
                                                rinv[:, hh:hh + 1])
                nc.scalar.dma_start_transpose(out=dst_sl,
                                              in_=ro.rearrange("p a b -> p (a b)"))

            proj_col(4, wc_k)
            wc_v = load_wcol(wq_r, 5, "wcv")
            for j in range(NKT):
                rope_rms(kraw[j], HKV, "k", csk_r[j], ssk_r[j], kt[:, :, j, :])
            proj_col(5, wc_v)
            for n in range(4):
                wc_q = load_wcol(wq_r, n, f"wcq{n}")
                proj_col(n, wc_q)
                # rope this column's 4 heads right away so DVE work overlaps
                # the next column's matmuls and attention can start early
                for qi in range(NQT):
                    rope_rms(qraw[qi][:, 4 * n:4 * n + 4, :], 4, "q",
                             csq_r[qi][:, n * 512:(n + 1) * 512],
                             ssq_r[qi][:, n * 512:(n + 1) * 512],
                             qt[:, 4 * n:4 * n + 4, qi, :])

            # band-edge masks (single DMA)
            msk = singles.tile([128, NQT, 2, 128], BF16)
            nc.gpsimd.dma_start(out=msk.rearrange("p a b c -> p (a b c)"),
                                in_=mask.ap().rearrange("i p m -> p i m"))

            # out-proj weight columns: streamed like qkv cols (2-buf rotation)
            wpc = [load_wcol(wp_r, n, f"wpc{n}") for n in range(4)]

            # ---------------- attention ----------------
            # yt per head: [128 dh, qi, 128 tok] bf16
            yt = [singles.tile([128, NQT, 128], BF16, name=f"yt{h}") for h in range(HQ)]
            for h in range(HQ):
                g = h // 4
                # scores per key tile jk, batched over the query tiles using it
                pes = {}
                for jk in range(NKT):
                    qlo, qhi = max(0, jk - 4), min(NQT - 1, jk)
                    nq = qhi - qlo + 1
                    sc = psA.tile([128, 512], F32, tag="sc", name="sc")
                    nc.tensor.matmul(sc[:, 0:nq * 128], kt[:, g, jk, :],
                                     qt[:, h, qlo:qhi + 1, :], start=True, stop=True)
                    pe = pepool.tile([128, 512], BF16, tag="pe", name="pe")
                    nc.scalar.activation(pe[:, 0:nq * 128], sc[:, 0:nq * 128],
                                         mybir.ActivationFunctionType.Exp,
                                         scale=float(SCALE))
                    # band-edge masks: diagonal block (qi == jk) and far block
                    # (qi == jk - 4)
                    if jk <= NQT - 1:
                        c0 = (jk - qlo) * 128
                        nc.vector.tensor_mul(pe[:, c0:c0 + 128], pe[:, c0:c0 + 128],
                                             msk[:, jk, 0, :])
                    if jk >= 4:
                        nc.vector.tensor_mul(pe[:, 0:128], pe[:, 0:128],
                                             msk[:, jk - 4, 1, :])
                    pes[jk] = (pe, qlo)
                ysb = work.tile([128, NQT, 128], BF16, tag="ysb", bufs=3)
                for qi in range(NQT):
                    yv = psB.tile([128, 512], F32, tag="yv")
                    for j in range(5):
                        jk = qi + j
                        pe, qlo = pes[jk]
                        c0 = (qi - qlo) * 128
                        nc.tensor.matmul(yv[:, 0:129], pe[:, c0:c0 + 128],
                                         vext[:, jk, g, 0:129],
                                         start=(j == 0), stop=(j == 4))
                    linv = work.tile([128, 1], F32, tag="linv", bufs=6)
                    nc.vector.reciprocal(linv, yv[:, 128:129])
                    nc.vector.tensor_scalar_mul(ysb[:, qi, :], yv[:, 0:128], linv)
                nc.scalar.dma_start_transpose(out=yt[h],
                                              in_=ysb.rearrange("p a b -> p (a b)"))

            # ---------------- output projection ----------------
            for n in range(4):
                for i in range(NQT):
                    po = pp_pool.tile([128, 512], F32, tag="pp", name="po")
                    for h in range(HQ):
                        nc.tensor.matmul(po, yt[h][:, i, :], wpc[n][:, h, :],
                                         start=(h == 0), stop=(h == 15))
                    osb = work.tile([128, 512], F32, tag="osb", bufs=3)
                    nc.scalar.copy(out=osb, in_=po)
                    nc.sync.dma_start(
                        out=out.ap()[i * 128:(i + 1) * 128, n * 512:(n + 1) * 512],
                        in_=osb)
    nc.compile()
    return nc


def _host_prep(x, w_qkv, w_proj):
    """Per-core input maps (numpy, bf16)."""
    bf = ml_dtypes.bfloat16
    # de-interleave perm within each Q/K head: new j <- old sigma(j)
    sig = np.empty(DH, np.int64)
    sig[:64] = np.arange(64) * 2
    sig[64:] = np.arange(64) * 2 + 1
    wqp = w_qkv.copy()
    for h in range(HQ + HKV):          # Q heads then K heads share layout
        base = h * DH
        wqp[base:base + DH] = w_qkv[base + sig]
    wqT = np.ascontiguousarray(wqp.T).astype(bf)          # [C, 3072]
    wpT = np.ascontiguousarray(w_proj.T).astype(bf)       # [C, C]

    inv_freq = 1.0 / (10000.0 ** (np.arange(0, DH, 2, dtype=np.float64) / DH))

    def tables(pos):  # pos [n] -> cos/sin [n, DH] (rotate-half, sign-baked sin)
        f = pos[:, None].astype(np.float64) * inv_freq[None, :]
        cs = np.concatenate([np.cos(f), np.cos(f)], axis=1)
        ss = np.concatenate([-np.sin(f), np.sin(f)], axis=1)
        return cs.astype(np.float32), ss.astype(np.float32)

    in_maps = []
    for core in range(8):
        b, c = divmod(core, 4)
        own0 = c * CHUNK
        w0 = own0 - CHUNK                     # window start (may be negative)
        xw = np.zeros((WTOK, C), np.float32)
        lo = max(0, w0)
        xw[lo - w0:, :] = x[b, lo:own0 + CHUNK, :]
        xtw = np.ascontiguousarray(xw.T).astype(bf)       # [C, 1024]

        kpos = np.maximum(np.arange(w0, own0 + CHUNK), 0)
        cskv, sskv = tables(kpos)                          # [1024, 128]
        csq1, ssq1 = tables(np.arange(own0, own0 + CHUNK))  # [512, 128]

        # triangle masks (j=0, j=4) in P^T layout: [qi, key_local_p, (2, query)]
        mask = np.zeros((NQT, 128, 256), np.float32)
        for i in range(NQT):
            qp = own0 + i * 128 + np.arange(128)[:, None]
            kp = w0 + i * 128 + np.arange(KWIN)[None, :]
            m_qk = ((kp >= 0) & (kp <= qp) & (qp - kp < WIN)).astype(np.float32)
            mt = m_qk.T.reshape(5, 128, 128)      # [j, k_local, q]
            mask[i] = np.concatenate([mt[0], mt[4]], axis=1)
        # [128, jk, g]: valid flag per key tile row, replicated over kv heads
        vldv = np.repeat((np.arange(w0, own0 + CHUNK) >= 0)
                         .astype(np.float32).reshape(NKT, 128).T[:, :, None],
                         HKV, axis=2).reshape(128, NKT * HKV)

        in_maps.append({
            "xt": xtw,
            "wq": wqT,
            "wp": wpT,
            "csq": np.tile(csq1, (1, HQ)).astype(bf),
            "ssq": np.tile(ssq1, (1, HQ)).astype(bf),
            "csk": np.tile(cskv, (1, HKV)).astype(bf),
            "ssk": np.tile(sskv, (1, HKV)).astype(bf),
            "mask": mask.astype(bf),
            "vld": vldv.astype(bf),
        })
    return in_maps


def kernel(x, w_qkv, w_proj, _trace=False):
    if _NC_CACHE[0] is None:
        _NC_CACHE[0] = _build_nc()
    nc = _NC_CACHE[0]
    in_maps = _host_prep(np.asarray(x, np.float32), np.asarray(w_qkv, np.float32),
                         np.asarray(w_proj, np.float32))
    res = run_bass_kernel_spmd(nc, in_maps, core_ids=list(range(8)), trace=_trace)
    outs = [res.results[i]["out"] for i in range(8)]
    full = np.empty((B, T, C), np.float32)
    for core in range(8):
        b, c = divmod(core, 4)
        full[b, c * CHUNK:(c + 1) * CHUNK] = outs[core]
    if _trace:
        kernel.last_exec_time_ns = res.exec_time_ns
        kernel.last_results = res
    return full


# revision 12
# speedup vs baseline: 1.0888x; 1.0888x over previous
"""Sliding-window GQA causal self-attention on 8 TRN2 NeuronCores.

Sharding: sequence-parallel. B=2 batches x 4 chunks of 512 tokens = 8 shards.
Sliding window (512) makes attention local: each chunk only needs the 512
preceding tokens (halo) for K/V, so there are NO collectives.

Per core: qkv proj (bf16, k-outer over 4 PSUM groups, quarter-column weight
streaming 8 deep so the PE starts ~4us in and never starves) -> rotate-half
RoPE + RMS-norm in row layout (rsqrt computed as exp(-0.5*ln(ms)) so the
whole kernel uses ONE activation table set: {ln, exp, copy} — no table
reloads; K's 1/rms * 1/sqrt(dh) is folded into the exp scale so K rows are
never explicitly normalized) -> DMA x-bar block transposes into [dh, tok]
layout (no PE transposes) -> banded attention (scores batched over query
tiles per key tile, exp'd probs feed attn@V as lhsT, softmax denominators
via a ones-column in V) -> output projection with streamed weight columns.
Column order Q0..Q3,K,V keeps all rope work clear of the attention phase.
Host pre-transposes x / weights, de-interleaves RoPE pairs into the qkv
weight so RoPE is a contiguous rotate-half, and precomputes cos/sin tables
and the multiplicative band mask per core.
"""

import sys

sys.path.insert(0, "/opt/trn_rl_repo")

import numpy as np
import ml_dtypes

import concourse.bass as bass
import concourse.mybir as mybir
import concourse.tile as tile
from concourse import bacc
from concourse.bass_utils import run_bass_kernel_spmd


def _install_ntff_hook():
    """antenv.axon_hooks is absent on this image; bridge the ctypes NTFF
    profiling hook from trn_agent_boot so trace=True works."""
    import types
    try:
        import antenv.axon_hooks  # noqa: F401
        return
    except ImportError:
        pass
    try:
        from trn_agent_boot.trn_boot import _ntff_profile_via_ctypes
        hook = _ntff_profile_via_ctypes("/opt/axon/libaxon_pjrt.so")
    except Exception:
        hook = None
    mod = types.ModuleType("antenv.axon_hooks")
    mod.get_axon_ntff_profile_hook = lambda: hook
    mod.set_axon_ntff_profile_hook = lambda h: None
    sys.modules["antenv.axon_hooks"] = mod


_install_ntff_hook()

BF16 = mybir.dt.bfloat16
F32 = mybir.dt.float32

B, T, C = 2, 2048, 2048
HQ, HKV, DH, WIN = 16, 4, 128, 512
RMS_EPS = 1.1920929e-07
CHUNK = 512          # own tokens per core
WTOK = 1024          # window tokens per core (halo 512 + own 512)
NKT = WTOK // 128    # 8 k-tiles
NQT = CHUNK // 128   # 4 q-tiles
KWIN = 640           # keys per q-tile (5 blocks of 128)
SCALE = 1.0 / np.sqrt(DH)
LN_SCALE = float(np.log(SCALE))

_NC_CACHE = [None]


def _build_nc():
    nc = bacc.Bacc("TRN2", target_bir_lowering=False, debug=False, num_devices=8)

    xt = nc.declare_dram_parameter("xt", [C, WTOK], BF16, False)        # x^T window
    wq = nc.declare_dram_parameter("wq", [C, (HQ + 2 * HKV) * DH], BF16, False)
    wp = nc.declare_dram_parameter("wp", [C, C], BF16, False)
    csq = nc.declare_dram_parameter("csq", [CHUNK, HQ * DH], BF16, False)
    ssq = nc.declare_dram_parameter("ssq", [CHUNK, HQ * DH], BF16, False)
    csk = nc.declare_dram_parameter("csk", [WTOK, HKV * DH], BF16, False)
    ssk = nc.declare_dram_parameter("ssk", [WTOK, HKV * DH], BF16, False)
    mask = nc.declare_dram_parameter("mask", [NQT, 128, 256], BF16, False)
    vld = nc.declare_dram_parameter("vld", [128, NKT * HKV], BF16, False)
    out = nc.declare_dram_parameter("out", [CHUNK, C], F32, True)

    # merged x^T chunks: 4 x [128, 4, 1024]
    xt_r = xt.ap().rearrange("(c k p) t -> c p k t", k=4, p=128)
    wq_r = wq.ap().rearrange("(k p) m -> p k m", p=128)       # [128, 16, 3072]
    wp_r = wp.ap().rearrange("(k p) m -> p k m", p=128)       # [128, 16, 2048]
    csq_r = csq.ap().rearrange("(n p) d -> n p d", p=128)     # 4 x [128,2048]
    ssq_r = ssq.ap().rearrange("(n p) d -> n p d", p=128)
    csk_r = csk.ap().rearrange("(n p) d -> n p d", p=128)     # 8 x [128,512]
    ssk_r = ssk.ap().rearrange("(n p) d -> n p d", p=128)

    Ln = mybir.ActivationFunctionType.Ln
    Exp = mybir.ActivationFunctionType.Exp

    with tile.TileContext(nc) as tc:
        with (
            tc.tile_pool(name="singles", bufs=1) as singles,
            tc.tile_pool(name="wtiles", bufs=8) as wtiles,      # weight quarter-cols
            tc.tile_pool(name="raw", bufs=1) as raw,            # Q/K raw + V
            tc.tile_pool(name="tbl", bufs=2) as tbl,            # rope tables (rotating)
            tc.tile_pool(name="work", bufs=2) as work,          # small DVE temps
            tc.tile_pool(name="pe", bufs=12) as pepool,         # exp'd probs
            tc.tile_pool(name="pp", bufs=4, space="PSUM") as pp_pool,   # proj (4 banks)
            tc.tile_pool(name="psA", bufs=2, space="PSUM") as psA,      # scores
            tc.tile_pool(name="psB", bufs=2, space="PSUM") as psB,      # AV out
        ):
            eps_t = singles.tile([128, 1], F32)
            nc.vector.memset(eps_t, RMS_EPS)
            lnsc_t = singles.tile([128, 1], F32)
            nc.vector.memset(lnsc_t, LN_SCALE)
            zero_t = singles.tile([128, 1], F32)
            nc.vector.memset(zero_t, 0.0)

            # x^T resident (4 merged tiles), first Q column interleaved so the
            # PE starts as soon as quarter 0 of both lands
            xts = [singles.tile([128, 4, WTOK], BF16, name=f"xts{c}") for c in range(4)]

            def load_wq4(col, c4, name):
                w4 = wtiles.tile([128, 4, 512], BF16, tag="wc4", name=name)
                nc.sync.dma_start(
                    out=w4, in_=wq_r[:, 4 * c4:4 * c4 + 4,
                                     col * 512:(col + 1) * 512])
                return w4

            wq0 = []
            for c4 in range(4):
                nc.sync.dma_start(out=xts[c4], in_=xt_r[c4])
                wq0.append(load_wq4(0, c4, f"w0_{c4}"))

            # raw (pre-rope) Q/K; V with ones column (softmax denominator)
            qraw = [raw.tile([128, HQ, DH], BF16, name=f"qraw{i}") for i in range(NQT)]
            kraw = [raw.tile([128, HKV, DH], BF16, name=f"kraw{j}") for j in range(NKT)]
            vext = raw.tile([128, NKT, HKV, 132], BF16, name="vext")
            nc.gpsimd.dma_start(out=vext[:, :, :, 128:129], in_=vld.ap())

            # per-key-tile exp scale: 1/rms(k) * 1/sqrt(dh), fp32 columns
            kinv = singles.tile([128, NKT, HKV], F32)
            # per-(qi,head) 1/rms(q)
            qinv = [singles.tile([128, HQ], F32, name=f"qinv{i}") for i in range(NQT)]

            # ---------------- QKV projection (k-outer, 4 psum groups) ------
            # feature columns: n=0..3 Q (heads 4n..4n+3), n=4 K, n=5 V
            def proj_col(n, wcs):
                itiles = list(range(4, 8)) if n < 4 else list(range(8))
                for half in range(len(itiles) // 4):
                    igrp = itiles[half * 4:half * 4 + 4]
                    pps = {i: pp_pool.tile([128, 512], F32, tag="pp", name=f"pp{n}_{i}")
                           for i in igrp}
                    for k in range(16):
                        for i in igrp:
                            nc.tensor.matmul(pps[i],
                                             xts[k // 4][:, k % 4, i * 128:(i + 1) * 128],
                                             wcs[k // 4][:, k % 4, :],
                                             start=(k == 0), stop=(k == 15))
                    for i in igrp:
                        if n < 4:
                            qi = i - 4
                            nc.scalar.copy(
                                out=qraw[qi][:, 4 * n:4 * n + 4, :].rearrange("p a b -> p (a b)"),
                                in_=pps[i][:])
                        elif n == 4:
                            nc.scalar.copy(
                                out=kraw[i][:, :, :].rearrange("p a b -> p (a b)"),
                                in_=pps[i][:])
                        else:
                            nc.scalar.copy(
                                out=vext[:, i, :, 0:128],
                                in_=pps[i].rearrange("p (g d) -> p g d", g=HKV))

            # ---------------- RoPE + RMS-norm (row layout) -----------------
            qt = singles.tile([128, HQ, NQT, 128], BF16)   # [dh, h, qi, tok]
            kt = singles.tile([128, HKV, NKT, 128], BF16)  # [dh, g, j, tok]
            # rope'd Q rows accumulate per qi (one block transpose per qi)
            roq = [singles.tile([128, HQ, DH], BF16, name=f"roq{i}") for i in range(NQT)]

            def rope(src, h, sfx, cs_src, ss_src, ro, rinv_out):
                # src: [128 tok, h, 128 dh] bf16 raw. Writes rope'd rows into
                # ro (unnormalized) and exp(-0.5*ln(ms+eps))[+lnscale] to
                # rinv_out ([128, h] f32). Norms commute with rotation.
                cs = tbl.tile([128, h, DH], BF16, tag=f"cs{sfx}", name=f"cs{sfx}")
                ss = tbl.tile([128, h, DH], BF16, tag=f"ss{sfx}", name=f"ss{sfx}")
                nc.gpsimd.dma_start(out=cs.rearrange("p a b -> p (a b)"), in_=cs_src)
                nc.gpsimd.dma_start(out=ss.rearrange("p a b -> p (a b)"), in_=ss_src)

                tmp = work.tile([128, h, DH], BF16, tag=f"tmp{sfx}")
                nc.vector.tensor_mul(tmp, src, src)
                ssum = work.tile([128, h], F32, tag=f"ssum{sfx}")
                nc.vector.reduce_sum(out=ssum, in_=tmp, axis=mybir.AxisListType.X)
                lns = work.tile([128, h], F32, tag=f"lns{sfx}")
                nc.scalar.activation(lns, ssum, Ln, bias=eps_t[:], scale=1.0 / DH)
                nc.scalar.activation(rinv_out, lns, Exp, scale=-0.5,
                                     bias=lnsc_t[:] if sfx == "k" else zero_t[:])

                nc.vector.tensor_mul(tmp[:, :, 0:64], src[:, :, 64:128], ss[:, :, 0:64])
                nc.vector.tensor_mul(tmp[:, :, 64:128], src[:, :, 0:64], ss[:, :, 64:128])
                nc.vector.tensor_mul(ro, src, cs)
                nc.vector.tensor_add(ro, ro, tmp)

            # Q columns first; their rope trails each column so everything is
            # transposed well before attention. K then V.
            for n in range(4):
                wcs = wq0 if n == 0 else [load_wq4(n, c4, f"w{n}_{c4}")
                                          for c4 in range(4)]
                proj_col(n, wcs)
                for qi in range(NQT):
                    ro_sl = roq[qi][:, 4 * n:4 * n + 4, :]
                    rope(qraw[qi][:, 4 * n:4 * n + 4, :], 4, "q",
                         csq_r[qi][:, n * 512:(n + 1) * 512],
                         ssq_r[qi][:, n * 512:(n + 1) * 512],
                         ro_sl, qinv[qi][:, 4 * n:4 * n + 4])
                    for hh in range(4):
                        nc.vector.tensor_scalar_mul(
                            ro_sl[:, hh, :], ro_sl[:, hh, :],
                            qinv[qi][:, 4 * n + hh:4 * n + hh + 1])
                    if n == 3:
                        nc.scalar.dma_start_transpose(
                            out=qt[:, :, qi, :],
                            in_=roq[qi].rearrange("p a b -> p (a b)"))

            wck = [load_wq4(4, c4, f"wk_{c4}") for c4 in range(4)]
            proj_col(4, wck)
            wcv = [load_wq4(5, c4, f"wv_{c4}") for c4 in range(4)]
            for j in range(NKT):
                rok = work.tile([128, HKV, DH], BF16, tag="rok", bufs=3)
                rope(kraw[j], HKV, "k", csk_r[j], ssk_r[j], rok, kinv[:, j, :])
                nc.scalar.dma_start_transpose(out=kt[:, :, j, :],
                                              in_=rok.rearrange("p a b -> p (a b)"))
            proj_col(5, wcv)

            # band-edge masks (single DMA)
            msk = singles.tile([128, NQT, 2, 128], BF16)
            nc.gpsimd.dma_start(out=msk.rearrange("p a b c -> p (a b c)"),
                                in_=mask.ap().rearrange("i p m -> p i m"))

            # out-proj weight columns, streamed through the same quarter pool
            wpc = []
            for n in range(4):
                wpc.append([wtiles.tile([128, 4, 512], BF16, tag="wc4", name=f"wp{n}_{c4}")
                            for c4 in range(4)])
                for c4 in range(4):
                    nc.sync.dma_start(
                        out=wpc[n][c4],
                        in_=wp_r[:, 4 * c4:4 * c4 + 4, n * 512:(n + 1) * 512])

            # ---------------- attention ----------------
            # yt per head: [128 dh, qi, 128 tok] bf16
            yt = [singles.tile([128, NQT, 128], BF16, name=f"yt{h}") for h in range(HQ)]
            for h in range(HQ):
                g = h // 4
                # scores per key tile jk, batched over the query tiles using it;
                # K's rms-norm and the 1/sqrt(dh) ride the exp scale (per key)
                pes = {}
                for jk in range(NKT):
                    qlo, qhi = max(0, jk - 4), min(NQT - 1, jk)
                    nq = qhi - qlo + 1
                    sc = psA.tile([128, 512], F32, tag="sc", name="sc")
                    nc.tensor.matmul(sc[:, 0:nq * 128], kt[:, g, jk, :],
                                     qt[:, h, qlo:qhi + 1, :], start=True, stop=True)
                    pe = pepool.tile([128, 512], BF16, tag="pe", name="pe")
                    nc.scalar.activation(pe[:, 0:nq * 128], sc[:, 0:nq * 128],
                                         Exp, scale=kinv[:, jk, g:g + 1])
                    # band-edge masks: diagonal block (qi == jk) and far block
                    # (qi == jk - 4)
                    if jk <= NQT - 1:
                        c0 = (jk - qlo) * 128
                        nc.vector.tensor_mul(pe[:, c0:c0 + 128], pe[:, c0:c0 + 128],
                                             msk[:, jk, 0, :])
                    if jk >= 4:
                        nc.vector.tensor_mul(pe[:, 0:128], pe[:, 0:128],
                                             msk[:, jk - 4, 1, :])
                    pes[jk] = (pe, qlo)
                ysb = work.tile([128, NQT, 128], BF16, tag="ysb", bufs=3)
                for qi in range(NQT):
                    yv = psB.tile([128, 256], F32, tag="yv")
                    for j in range(5):
                        jk = qi + j
                        pe, qlo = pes[jk]
                        c0 = (qi - qlo) * 128
                        nc.tensor.matmul(yv[:, 0:129], pe[:, c0:c0 + 128],
                                         vext[:, jk, g, 0:129],
                                         start=(j == 0), stop=(j == 4))
                    linv = work.tile([128, 1], F32, tag="linv", bufs=6)
                    nc.vector.reciprocal(linv, yv[:, 128:129])
                    nc.vector.tensor_scalar_mul(ysb[:, qi, :], yv[:, 0:128], linv)
                nc.scalar.dma_start_transpose(out=yt[h],
                                              in_=ysb.rearrange("p a b -> p (a b)"))

            # ---------------- output projection ----------------
            for n in range(4):
                for i in range(NQT):
                    po = pp_pool.tile([128, 512], F32, tag="pp", name="po")
                    for h in range(HQ):
                        nc.tensor.matmul(po, yt[h][:, i, :], wpc[n][h // 4][:, h % 4, :],
                                         start=(h == 0), stop=(h == 15))
                    osb = work.tile([128, 512], F32, tag="osb", bufs=3)
                    nc.scalar.copy(out=osb, in_=po)
                    nc.sync.dma_start(
                        out=out.ap()[i * 128:(i + 1) * 128, n * 512:(n + 1) * 512],
                        in_=osb)
    nc.compile()
    return nc


def _host_prep(x, w_qkv, w_proj):
    """Per-core input maps (numpy, bf16)."""
    bf = ml_dtypes.bfloat16
    # de-interleave perm within each Q/K head: new j <- old sigma(j)
    sig = np.empty(DH, np.int64)
    sig[:64] = np.arange(64) * 2
    sig[64:] = np.arange(64) * 2 + 1
    wqp = w_qkv.copy()
    for h in range(HQ + HKV):          # Q heads then K heads share layout
        base = h * DH
        wqp[base:base + DH] = w_qkv[base + sig]
    wqT = np.ascontiguousarray(wqp.T).astype(bf)          # [C, 3072]
    wpT = np.ascontiguousarray(w_proj.T).astype(bf)       # [C, C]

    inv_freq = 1.0 / (10000.0 ** (np.arange(0, DH, 2, dtype=np.float64) / DH))

    def tables(pos):  # pos [n] -> cos/sin [n, DH] (rotate-half, sign-baked sin)
        f = pos[:, None].astype(np.float64) * inv_freq[None, :]
        cs = np.concatenate([np.cos(f), np.cos(f)], axis=1)
        ss = np.concatenate([-np.sin(f), np.sin(f)], axis=1)
        return cs.astype(np.float32), ss.astype(np.float32)

    in_maps = []
    for core in range(8):
        b, c = divmod(core, 4)
        own0 = c * CHUNK
        w0 = own0 - CHUNK                     # window start (may be negative)
        xw = np.zeros((WTOK, C), np.float32)
        lo = max(0, w0)
        xw[lo - w0:, :] = x[b, lo:own0 + CHUNK, :]
        xtw = np.ascontiguousarray(xw.T).astype(bf)       # [C, 1024]

        kpos = np.maximum(np.arange(w0, own0 + CHUNK), 0)
        cskv, sskv = tables(kpos)                          # [1024, 128]
        csq1, ssq1 = tables(np.arange(own0, own0 + CHUNK))  # [512, 128]

        # triangle masks (j=0, j=4) in P^T layout: [qi, key_local_p, (2, query)]
        mask = np.zeros((NQT, 128, 256), np.float32)
        for i in range(NQT):
            qp = own0 + i * 128 + np.arange(128)[:, None]
            kp = w0 + i * 128 + np.arange(KWIN)[None, :]
            m_qk = ((kp >= 0) & (kp <= qp) & (qp - kp < WIN)).astype(np.float32)
            mt = m_qk.T.reshape(5, 128, 128)      # [j, k_local, q]
            mask[i] = np.concatenate([mt[0], mt[4]], axis=1)
        # [128, jk, g]: valid flag per key tile row, replicated over kv heads
        vldv = np.repeat((np.arange(w0, own0 + CHUNK) >= 0)
                         .astype(np.float32).reshape(NKT, 128).T[:, :, None],
                         HKV, axis=2).reshape(128, NKT * HKV)

        in_maps.append({
            "xt": xtw,
            "wq": wqT,
            "wp": wpT,
            "csq": np.tile(csq1, (1, HQ)).astype(bf),
            "ssq": np.tile(ssq1, (1, HQ)).astype(bf),
            "csk": np.tile(cskv, (1, HKV)).astype(bf),
            "ssk": np.tile(sskv, (1, HKV)).astype(bf),
            "mask": mask.astype(bf),
            "vld": vldv.astype(bf),
        })
    return in_maps


def kernel(x, w_qkv, w_proj, _trace=False):
    if _NC_CACHE[0] is None:
        _NC_CACHE[0] = _build_nc()
    nc = _NC_CACHE[0]
    in_maps = _host_prep(np.asarray(x, np.float32), np.asarray(w_qkv, np.float32),
                         np.asarray(w_proj, np.float32))
    res = run_bass_kernel_spmd(nc, in_maps, core_ids=list(range(8)), trace=_trace)
    outs = [res.results[i]["out"] for i in range(8)]
    full = np.empty((B, T, C), np.float32)
    for core in range(8):
        b, c = divmod(core, 4)
        full[b, c * CHUNK:(c + 1) * CHUNK] = outs[core]
    if _trace:
        kernel.last_exec_time_ns = res.exec_time_ns
        kernel.last_results = res
    return full


# revision 17
# speedup vs baseline: 1.1036x; 1.0136x over previous
"""Sliding-window GQA causal self-attention on 8 TRN2 NeuronCores.

Sharding: sequence-parallel. B=2 batches x 4 chunks of 512 tokens = 8 shards.
Sliding window (512) makes attention local: each chunk only needs the 512
preceding tokens (halo) for K/V, so there are NO collectives.

Per core: qkv proj (bf16, k-outer over 4 PSUM groups, quarter-column weight
streaming 8 deep so the PE starts ~4us in and never starves) -> rotate-half
RoPE + RMS-norm in row layout (rsqrt computed as exp(-0.5*ln(ms)) so the
whole kernel uses ONE activation table set: {ln, exp, copy} — no table
reloads; K's 1/rms * 1/sqrt(dh) is folded into the exp scale so K rows are
never explicitly normalized) -> DMA x-bar block transposes into [dh, tok]
layout (no PE transposes) -> banded attention (scores batched over query
tiles per key tile, exp'd probs feed attn@V as lhsT, softmax denominators
via a ones-column in V) -> output projection with streamed weight columns.
Column order Q0..Q3,K,V keeps all rope work clear of the attention phase.
Host pre-transposes x / weights, de-interleaves RoPE pairs into the qkv
weight so RoPE is a contiguous rotate-half, and precomputes cos/sin tables
and the multiplicative band mask per core.
"""

import sys

sys.path.insert(0, "/opt/trn_rl_repo")

import numpy as np
import ml_dtypes

import concourse.bass as bass
import concourse.mybir as mybir
import concourse.tile as tile
from concourse import bacc
from concourse.bass_utils import run_bass_kernel_spmd


def _install_ntff_hook():
    """antenv.axon_hooks is absent on this image; bridge the ctypes NTFF
    profiling hook from trn_agent_boot so trace=True works."""
    import types
    try:
        import antenv.axon_hooks  # noqa: F401
        return
    except ImportError:
        pass
    try:
        from trn_agent_boot.trn_boot import _ntff_profile_via_ctypes
        hook = _ntff_profile_via_ctypes("/opt/axon/libaxon_pjrt.so")
    except Exception:
        hook = None
    mod = types.ModuleType("antenv.axon_hooks")
    mod.get_axon_ntff_profile_hook = lambda: hook
    mod.set_axon_ntff_profile_hook = lambda h: None
    sys.modules["antenv.axon_hooks"] = mod


_install_ntff_hook()

BF16 = mybir.dt.bfloat16
F32 = mybir.dt.float32

B, T, C = 2, 2048, 2048
HQ, HKV, DH, WIN = 16, 4, 128, 512
RMS_EPS = 1.1920929e-07
CHUNK = 512          # own tokens per core
WTOK = 1024          # window tokens per core (halo 512 + own 512)
NKT = WTOK // 128    # 8 k-tiles
NQT = CHUNK // 128   # 4 q-tiles
KWIN = 640           # keys per q-tile (5 blocks of 128)
SCALE = 1.0 / np.sqrt(DH)
LN_SCALE = float(np.log(SCALE))

_NC_CACHE = [None]


def _build_nc():
    nc = bacc.Bacc("TRN2", target_bir_lowering=False, debug=False, num_devices=8)

    xt = nc.declare_dram_parameter("xt", [C, WTOK], BF16, False)        # x^T window
    wq = nc.declare_dram_parameter("wq", [C, (HQ + 2 * HKV) * DH], BF16, False)
    wp = nc.declare_dram_parameter("wp", [C, C], BF16, False)
    csq = nc.declare_dram_parameter("csq", [CHUNK, HQ * DH], BF16, False)
    ssq = nc.declare_dram_parameter("ssq", [CHUNK, HQ * DH], BF16, False)
    csk = nc.declare_dram_parameter("csk", [WTOK, HKV * DH], BF16, False)
    ssk = nc.declare_dram_parameter("ssk", [WTOK, HKV * DH], BF16, False)
    mask = nc.declare_dram_parameter("mask", [NQT, 128, 256], BF16, False)
    vld = nc.declare_dram_parameter("vld", [128, NKT * HKV], BF16, False)
    out = nc.declare_dram_parameter("out", [CHUNK, C], F32, True)

    # merged x^T chunks: 4 x [128, 4, 1024]
    xt_r = xt.ap().rearrange("(c k p) t -> c p k t", k=4, p=128)
    wq_r = wq.ap().rearrange("(k p) m -> p k m", p=128)       # [128, 16, 3072]
    wp_r = wp.ap().rearrange("(k p) m -> p k m", p=128)       # [128, 16, 2048]
    csq_r = csq.ap().rearrange("(n p) d -> n p d", p=128)     # 4 x [128,2048]
    ssq_r = ssq.ap().rearrange("(n p) d -> n p d", p=128)
    csk_r = csk.ap().rearrange("(n p) d -> n p d", p=128)     # 8 x [128,512]
    ssk_r = ssk.ap().rearrange("(n p) d -> n p d", p=128)

    Exp = mybir.ActivationFunctionType.Exp
    Sqrt = mybir.ActivationFunctionType.Sqrt

    with tile.TileContext(nc) as tc:
        with (
            tc.tile_pool(name="singles", bufs=1) as singles,
            tc.tile_pool(name="wtiles", bufs=8) as wtiles,      # weight quarter-cols
            tc.tile_pool(name="raw", bufs=1) as raw,            # Q/K raw + V
            tc.tile_pool(name="tbl", bufs=2) as tbl,            # rope tables (rotating)
            tc.tile_pool(name="work", bufs=2) as work,          # small DVE temps
            tc.tile_pool(name="pe", bufs=12) as pepool,         # exp'd probs
            tc.tile_pool(name="pp", bufs=4, space="PSUM") as pp_pool,   # proj (4 banks)
            tc.tile_pool(name="psA", bufs=2, space="PSUM") as psA,      # scores
            tc.tile_pool(name="psB", bufs=2, space="PSUM") as psB,      # AV out
        ):
            eps_t = singles.tile([128, 1], F32)
            nc.vector.memset(eps_t, RMS_EPS)
            # K-side: kinv = 1/sqrt(dh) * rsqrt(ssum/dh + eps) = rsqrt(ssum + dh*eps)
            epsk_t = singles.tile([128, 1], F32)
            nc.vector.memset(epsk_t, DH * RMS_EPS)

            # x^T resident (4 merged tiles), first Q column interleaved so the
            # PE starts as soon as quarter 0 of both lands
            xts = [singles.tile([128, 4, WTOK], BF16, name=f"xts{c}") for c in range(4)]

            def load_wq4(col, c4, name):
                w4 = wtiles.tile([128, 4, 512], BF16, tag="wc4", name=name)
                nc.sync.dma_start(
                    out=w4, in_=wq_r[:, 4 * c4:4 * c4 + 4,
                                     col * 512:(col + 1) * 512])
                return w4

            wq0 = []
            for c4 in range(4):
                nc.sync.dma_start(out=xts[c4], in_=xt_r[c4])
                wq0.append(load_wq4(0, c4, f"w0_{c4}"))

            # raw (pre-rope) Q/K; V with ones column (softmax denominator)
            qraw = [raw.tile([128, HQ, DH], BF16, name=f"qraw{i}") for i in range(NQT)]
            kraw = [raw.tile([128, HKV, DH], BF16, name=f"kraw{j}") for j in range(NKT)]
            vext = raw.tile([128, NKT, HKV, 132], BF16, name="vext")
            nc.gpsimd.dma_start(out=vext[:, :, :, 128:129], in_=vld.ap())

            # per-key-tile exp scale: 1/rms(k) * 1/sqrt(dh), fp32 columns
            kinv = singles.tile([128, NKT, HKV], F32)
            # per-(qi,head) 1/rms(q)
            qinv = [singles.tile([128, HQ], F32, name=f"qinv{i}") for i in range(NQT)]

            # ---------------- QKV projection (k-outer, 4 psum groups) ------
            # feature columns: n=0..3 Q (heads 4n..4n+3), n=4 K, n=5 V
            def proj_col(n, wcs):
                itiles = list(range(4, 8)) if n < 4 else list(range(8))
                for half in range(len(itiles) // 4):
                    igrp = itiles[half * 4:half * 4 + 4]
                    pps = {i: pp_pool.tile([128, 512], F32, tag="pp", name=f"pp{n}_{i}")
                           for i in igrp}
                    for k in range(16):
                        for i in igrp:
                            nc.tensor.matmul(pps[i],
                                             xts[k // 4][:, k % 4, i * 128:(i + 1) * 128],
                                             wcs[k // 4][:, k % 4, :],
                                             start=(k == 0), stop=(k == 15))
                    for i in igrp:
                        if n < 4:
                            qi = i - 4
                            nc.scalar.copy(
                                out=qraw[qi][:, 4 * n:4 * n + 4, :].rearrange("p a b -> p (a b)"),
                                in_=pps[i][:])
                        elif n == 4:
                            nc.scalar.copy(
                                out=kraw[i][:, :, :].rearrange("p a b -> p (a b)"),
                                in_=pps[i][:])
                        else:
                            nc.scalar.copy(
                                out=vext[:, i, :, 0:128],
                                in_=pps[i].rearrange("p (g d) -> p g d", g=HKV))

            # ---------------- RoPE + RMS-norm (row layout) -----------------
            qt = singles.tile([128, HQ, NQT, 128], BF16)   # [dh, h, qi, tok]
            kt = singles.tile([128, HKV, NKT, 128], BF16)  # [dh, g, j, tok]
            # rope'd Q rows accumulate per qi (one block transpose per qi)
            roq = [singles.tile([128, HQ, DH], BF16, name=f"roq{i}") for i in range(NQT)]

            def rope(src, h, sfx, cs_src, ss_src, ro, rinv_out):
                # src: [128 tok, h, 128 dh] bf16 raw. Writes rope'd rows into
                # ro (unnormalized) and exp(-0.5*ln(ms+eps))[+lnscale] to
                # rinv_out ([128, h] f32). Norms commute with rotation.
                cs = tbl.tile([128, h, DH], BF16, tag=f"cs{sfx}", name=f"cs{sfx}")
                ss = tbl.tile([128, h, DH], BF16, tag=f"ss{sfx}", name=f"ss{sfx}")
                nc.gpsimd.dma_start(out=cs.rearrange("p a b -> p (a b)"), in_=cs_src)
                nc.gpsimd.dma_start(out=ss.rearrange("p a b -> p (a b)"), in_=ss_src)

                tmp = work.tile([128, h, DH], BF16, tag=f"tmp{sfx}")
                nc.vector.tensor_mul(tmp, src, src)
                ssum = work.tile([128, h], F32, tag=f"ssum{sfx}")
                nc.vector.reduce_sum(out=ssum, in_=tmp, axis=mybir.AxisListType.X)
                rstd = work.tile([128, h], F32, tag=f"rstd{sfx}")
                if sfx == "k":
                    nc.scalar.activation(rstd, ssum, Sqrt, bias=epsk_t[:], scale=1.0)
                else:
                    nc.scalar.activation(rstd, ssum, Sqrt, bias=eps_t[:], scale=1.0 / DH)
                nc.vector.reciprocal(rinv_out, rstd)

                nc.vector.tensor_mul(tmp[:, :, 0:64], src[:, :, 64:128], ss[:, :, 0:64])
                nc.vector.tensor_mul(tmp[:, :, 64:128], src[:, :, 0:64], ss[:, :, 64:128])
                nc.vector.tensor_mul(ro, src, cs)
                nc.vector.tensor_add(ro, ro, tmp)

            # Q columns first; their rope trails each column so everything is
            # transposed well before attention. K then V.
            for n in range(4):
                wcs = wq0 if n == 0 else [load_wq4(n, c4, f"w{n}_{c4}")
                                          for c4 in range(4)]
                proj_col(n, wcs)
                for qi in range(NQT):
                    ro_sl = roq[qi][:, 4 * n:4 * n + 4, :]
                    rope(qraw[qi][:, 4 * n:4 * n + 4, :], 4, "q",
                         csq_r[qi][:, n * 512:(n + 1) * 512],
                         ssq_r[qi][:, n * 512:(n + 1) * 512],
                         ro_sl, qinv[qi][:, 4 * n:4 * n + 4])
                    for hh in range(4):
                        nc.vector.tensor_scalar_mul(
                            ro_sl[:, hh, :], ro_sl[:, hh, :],
                            qinv[qi][:, 4 * n + hh:4 * n + hh + 1])
                    if n == 3:
                        nc.scalar.dma_start_transpose(
                            out=qt[:, :, qi, :],
                            in_=roq[qi].rearrange("p a b -> p (a b)"))

            wck = [load_wq4(4, c4, f"wk_{c4}") for c4 in range(4)]
            proj_col(4, wck)
            wcv = [load_wq4(5, c4, f"wv_{c4}") for c4 in range(4)]
            for j in range(NKT):
                rok = work.tile([128, HKV, DH], BF16, tag="rok", bufs=3)
                rope(kraw[j], HKV, "k", csk_r[j], ssk_r[j], rok, kinv[:, j, :])
                nc.scalar.dma_start_transpose(out=kt[:, :, j, :],
                                              in_=rok.rearrange("p a b -> p (a b)"))
            proj_col(5, wcv)

            # band-edge masks (single DMA)
            msk = singles.tile([128, NQT, 2, 128], BF16)
            nc.gpsimd.dma_start(out=msk.rearrange("p a b c -> p (a b c)"),
                                in_=mask.ap().rearrange("i p m -> p i m"))

            # out-proj weight columns, streamed through the same quarter pool
            wpc = []
            for n in range(4):
                wpc.append([wtiles.tile([128, 4, 512], BF16, tag="wc4", name=f"wp{n}_{c4}")
                            for c4 in range(4)])
                for c4 in range(4):
                    nc.sync.dma_start(
                        out=wpc[n][c4],
                        in_=wp_r[:, 4 * c4:4 * c4 + 4, n * 512:(n + 1) * 512])

            # ---------------- attention ----------------
            # yt per head: [128 dh, qi, 128 tok] bf16
            yt = [singles.tile([128, NQT, 128], BF16, name=f"yt{h}") for h in range(HQ)]
            for h in range(HQ):
                g = h // 4
                # scores per key tile jk, batched over the query tiles using it;
                # K's rms-norm and the 1/sqrt(dh) ride the exp scale (per key)
                pes = {}
                for jk in range(NKT):
                    qlo, qhi = max(0, jk - 4), min(NQT - 1, jk)
                    nq = qhi - qlo + 1
                    sc = psA.tile([128, 512], F32, tag="sc", name="sc")
                    nc.tensor.matmul(sc[:, 0:nq * 128], kt[:, g, jk, :],
                                     qt[:, h, qlo:qhi + 1, :], start=True, stop=True)
                    pe = pepool.tile([128, 512], BF16, tag="pe", name="pe")
                    nc.scalar.activation(pe[:, 0:nq * 128], sc[:, 0:nq * 128],
                                         Exp, scale=kinv[:, jk, g:g + 1])
                    # band-edge masks: diagonal block (qi == jk) and far block
                    # (qi == jk - 4)
                    if jk <= NQT - 1:
                        c0 = (jk - qlo) * 128
                        nc.vector.tensor_mul(pe[:, c0:c0 + 128], pe[:, c0:c0 + 128],
                                             msk[:, jk, 0, :])
                    if jk >= 4:
                        nc.vector.tensor_mul(pe[:, 0:128], pe[:, 0:128],
                                             msk[:, jk - 4, 1, :])
                    pes[jk] = (pe, qlo)
                ysb = work.tile([128, NQT, 128], BF16, tag="ysb", bufs=3)
                for qi in range(NQT):
                    yv = psB.tile([128, 256], F32, tag="yv")
                    for j in range(5):
                        jk = qi + j
                        pe, qlo = pes[jk]
                        c0 = (qi - qlo) * 128
                        nc.tensor.matmul(yv[:, 0:129], pe[:, c0:c0 + 128],
                                         vext[:, jk, g, 0:129],
                                         start=(j == 0), stop=(j == 4))
                    linv = work.tile([128, 1], F32, tag="linv", bufs=6)
                    nc.vector.reciprocal(linv, yv[:, 128:129])
                    nc.vector.tensor_scalar_mul(ysb[:, qi, :], yv[:, 0:128], linv)
                nc.scalar.dma_start_transpose(out=yt[h],
                                              in_=ysb.rearrange("p a b -> p (a b)"))

            # ---------------- output projection ----------------
            for n in range(4):
                for i in range(NQT):
                    po = pp_pool.tile([128, 512], F32, tag="pp", name="po")
                    for h in range(HQ):
                        nc.tensor.matmul(po, yt[h][:, i, :], wpc[n][h // 4][:, h % 4, :],
                                         start=(h == 0), stop=(h == 15))
                    osb = work.tile([128, 512], F32, tag="osb", bufs=3)
                    nc.scalar.copy(out=osb, in_=po)
                    nc.sync.dma_start(
                        out=out.ap()[i * 128:(i + 1) * 128, n * 512:(n + 1) * 512],
                        in_=osb)
    nc.compile()
    return nc


def _host_prep(x, w_qkv, w_proj):
    """Per-core input maps (numpy, bf16)."""
    bf = ml_dtypes.bfloat16
    # de-interleave perm within each Q/K head: new j <- old sigma(j)
    sig = np.empty(DH, np.int64)
    sig[:64] = np.arange(64) * 2
    sig[64:] = np.arange(64) * 2 + 1
    wqp = w_qkv.copy()
    for h in range(HQ + HKV):          # Q heads then K heads share layout
        base = h * DH
        wqp[base:base + DH] = w_qkv[base + sig]
    wqT = np.ascontiguousarray(wqp.T).astype(bf)          # [C, 3072]
    wpT = np.ascontiguousarray(w_proj.T).astype(bf)       # [C, C]

    inv_freq = 1.0 / (10000.0 ** (np.arange(0, DH, 2, dtype=np.float64) / DH))

    def tables(pos):  # pos [n] -> cos/sin [n, DH] (rotate-half, sign-baked sin)
        f = pos[:, None].astype(np.float64) * inv_freq[None, :]
        cs = np.concatenate([np.cos(f), np.cos(f)], axis=1)
        ss = np.concatenate([-np.sin(f), np.sin(f)], axis=1)
        return cs.astype(np.float32), ss.astype(np.float32)

    in_maps = []
    for core in range(8):
        b, c = divmod(core, 4)
        own0 = c * CHUNK
        w0 = own0 - CHUNK                     # window start (may be negative)
        xw = np.zeros((WTOK, C), np.float32)
        lo = max(0, w0)
        xw[lo - w0:, :] = x[b, lo:own0 + CHUNK, :]
        xtw = np.ascontiguousarray(xw.T).astype(bf)       # [C, 1024]

        kpos = np.maximum(np.arange(w0, own0 + CHUNK), 0)
        cskv, sskv = tables(kpos)                          # [1024, 128]
        csq1, ssq1 = tables(np.arange(own0, own0 + CHUNK))  # [512, 128]

        # triangle masks (j=0, j=4) in P^T layout: [qi, key_local_p, (2, query)]
        mask = np.zeros((NQT, 128, 256), np.float32)
        for i in range(NQT):
            qp = own0 + i * 128 + np.arange(128)[:, None]
            kp = w0 + i * 128 + np.arange(KWIN)[None, :]
            m_qk = ((kp >= 0) & (kp <= qp) & (qp - kp < WIN)).astype(np.float32)
            mt = m_qk.T.reshape(5, 128, 128)      # [j, k_local, q]
            mask[i] = np.concatenate([mt[0], mt[4]], axis=1)
        # [128, jk, g]: valid flag per key tile row, replicated over kv heads
        vldv = np.repeat((np.arange(w0, own0 + CHUNK) >= 0)
                         .astype(np.float32).reshape(NKT, 128).T[:, :, None],
                         HKV, axis=2).reshape(128, NKT * HKV)

        in_maps.append({
            "xt": xtw,
            "wq": wqT,
            "wp": wpT,
            "csq": np.tile(csq1, (1, HQ)).astype(bf),
            "ssq": np.tile(ssq1, (1, HQ)).astype(bf),
            "csk": np.tile(cskv, (1, HKV)).astype(bf),
            "ssk": np.tile(sskv, (1, HKV)).astype(bf),
            "mask": mask.astype(bf),
            "vld": vldv.astype(bf),
        })
    return in_maps


def kernel(x, w_qkv, w_proj, _trace=False):
    if _NC_CACHE[0] is None:
        _NC_CACHE[0] = _build_nc()
    nc = _NC_CACHE[0]
    in_maps = _host_prep(np.asarray(x, np.float32), np.asarray(w_qkv, np.float32),
                         np.asarray(w_proj, np.float32))
    res = run_bass_kernel_spmd(nc, in_maps, core_ids=list(range(8)), trace=_trace)
    outs = [res.results[i]["out"] for i in range(8)]
    full = np.empty((B, T, C), np.float32)
    for core in range(8):
        b, c = divmod(core, 4)
        full[b, c * CHUNK:(c + 1) * CHUNK] = outs[core]
    if _trace:
        kernel.last_exec_time_ns = res.exec_time_ns
        kernel.last_results = res
    return full


# revision 20
# speedup vs baseline: 1.3582x; 1.2307x over previous
"""Sliding-window GQA causal self-attention on 8 TRN2 NeuronCores.

Sharding: sequence-parallel. B=2 batches x 4 chunks of 512 tokens = 8 shards.
Sliding window (512) makes attention local: each chunk only needs the 512
preceding tokens (halo) for K/V, so there are NO collectives.

Per core: qkv proj (bf16, k-outer over 4 PSUM groups, quarter-column weight
streaming 8 deep so the PE starts ~4us in and never starves) -> rotate-half
RoPE + RMS-norm in row layout (rsqrt computed as exp(-0.5*ln(ms)) so the
whole kernel uses ONE activation table set: {ln, exp, copy} — no table
reloads; K's 1/rms * 1/sqrt(dh) is folded into the exp scale so K rows are
never explicitly normalized) -> DMA x-bar block transposes into [dh, tok]
layout (no PE transposes) -> banded attention (scores batched over query
tiles per key tile, exp'd probs feed attn@V as lhsT, softmax denominators
via a ones-column in V) -> output projection with streamed weight columns.
Column order Q0..Q3,K,V keeps all rope work clear of the attention phase.
Host pre-transposes x / weights, de-interleaves RoPE pairs into the qkv
weight so RoPE is a contiguous rotate-half, and precomputes cos/sin tables
and the multiplicative band mask per core.
"""

import sys

sys.path.insert(0, "/opt/trn_rl_repo")

import numpy as np
import ml_dtypes

import concourse.bass as bass
import concourse.mybir as mybir
import concourse.tile as tile
from concourse import bacc
from concourse.bass_utils import run_bass_kernel_spmd


def _install_ntff_hook():
    """antenv.axon_hooks is absent on this image; bridge the ctypes NTFF
    profiling hook from trn_agent_boot so trace=True works."""
    import types
    try:
        import antenv.axon_hooks  # noqa: F401
        return
    except ImportError:
        pass
    try:
        from trn_agent_boot.trn_boot import _ntff_profile_via_ctypes
        hook = _ntff_profile_via_ctypes("/opt/axon/libaxon_pjrt.so")
    except Exception:
        hook = None
    mod = types.ModuleType("antenv.axon_hooks")
    mod.get_axon_ntff_profile_hook = lambda: hook
    mod.set_axon_ntff_profile_hook = lambda h: None
    sys.modules["antenv.axon_hooks"] = mod


_install_ntff_hook()

BF16 = mybir.dt.bfloat16
F32 = mybir.dt.float32

B, T, C = 2, 2048, 2048
HQ, HKV, DH, WIN = 16, 4, 128, 512
RMS_EPS = 1.1920929e-07
CHUNK = 512          # own tokens per core
WTOK = 1024          # window tokens per core (halo 512 + own 512)
NKT = WTOK // 128    # 8 k-tiles
NQT = CHUNK // 128   # 4 q-tiles
KWIN = 640           # keys per q-tile (5 blocks of 128)
SCALE = 1.0 / np.sqrt(DH)
LN_SCALE = float(np.log(SCALE))

_NC_CACHE = [None]


def _build_nc():
    nc = bacc.Bacc("TRN2", target_bir_lowering=False, debug=False, num_devices=8)

    xt = nc.declare_dram_parameter("xt", [C, WTOK], BF16, False)        # x^T window
    wq = nc.declare_dram_parameter("wq", [C, (HQ + 2 * HKV) * DH], BF16, False)
    wp = nc.declare_dram_parameter("wp", [C, C], BF16, False)
    csq = nc.declare_dram_parameter("csq", [CHUNK, HQ * DH], BF16, False)
    ssq = nc.declare_dram_parameter("ssq", [CHUNK, HQ * DH], BF16, False)
    csk = nc.declare_dram_parameter("csk", [WTOK, HKV * DH], BF16, False)
    ssk = nc.declare_dram_parameter("ssk", [WTOK, HKV * DH], BF16, False)
    mask = nc.declare_dram_parameter("mask", [NQT, 128, 256], BF16, False)
    vld = nc.declare_dram_parameter("vld", [128, NKT * HKV], BF16, False)
    out = nc.declare_dram_parameter("out", [CHUNK, C], F32, True)

    # merged x^T chunks: 4 x [128, 4, 1024]
    xt_r = xt.ap().rearrange("(c k p) t -> c p k t", k=4, p=128)
    wq_r = wq.ap().rearrange("(k p) m -> p k m", p=128)       # [128, 16, 3072]
    wp_r = wp.ap().rearrange("(k p) m -> p k m", p=128)       # [128, 16, 2048]
    csq_r = csq.ap().rearrange("(n p) d -> n p d", p=128)     # 4 x [128,2048]
    ssq_r = ssq.ap().rearrange("(n p) d -> n p d", p=128)
    csk_r = csk.ap().rearrange("(n p) d -> n p d", p=128)     # 8 x [128,512]
    ssk_r = ssk.ap().rearrange("(n p) d -> n p d", p=128)

    Exp = mybir.ActivationFunctionType.Exp
    Sqrt = mybir.ActivationFunctionType.Sqrt

    with tile.TileContext(nc) as tc:
        with (
            tc.tile_pool(name="singles", bufs=1) as singles,
            tc.tile_pool(name="wtiles", bufs=8) as wtiles,      # weight quarter-cols
            tc.tile_pool(name="raw", bufs=1) as raw,            # Q/K raw + V
            tc.tile_pool(name="tbl", bufs=2) as tbl,            # rope tables (rotating)
            tc.tile_pool(name="work", bufs=2) as work,          # small DVE temps
            tc.tile_pool(name="pe", bufs=16) as pepool,         # exp'd probs
            tc.tile_pool(name="pp", bufs=4, space="PSUM") as pp_pool,   # proj (4 banks)
            tc.tile_pool(name="psA", bufs=2, space="PSUM") as psA,      # scores
            tc.tile_pool(name="psB", bufs=2, space="PSUM") as psB,      # AV out
        ):
            eps_t = singles.tile([128, 1], F32)
            nc.vector.memset(eps_t, RMS_EPS)
            # K-side: kinv = 1/sqrt(dh) * rsqrt(ssum/dh + eps) = rsqrt(ssum + dh*eps)
            epsk_t = singles.tile([128, 1], F32)
            nc.vector.memset(epsk_t, DH * RMS_EPS)

            # x^T resident (4 merged tiles), first Q column interleaved so the
            # PE starts as soon as quarter 0 of both lands
            xts = [singles.tile([128, 4, WTOK], BF16, name=f"xts{c}") for c in range(4)]

            def load_wq4(col, c4, name):
                w4 = wtiles.tile([128, 4, 512], BF16, tag="wc4", name=name)
                nc.sync.dma_start(
                    out=w4, in_=wq_r[:, 4 * c4:4 * c4 + 4,
                                     col * 512:(col + 1) * 512])
                return w4

            wq0 = []
            for c4 in range(4):
                nc.sync.dma_start(out=xts[c4], in_=xt_r[c4])
                wq0.append(load_wq4(0, c4, f"w0_{c4}"))

            # raw (pre-rope) Q/K; V with ones column (softmax denominator)
            qraw = [raw.tile([128, HQ, DH], BF16, name=f"qraw{i}") for i in range(NQT)]
            kraw = [raw.tile([128, HKV, DH], BF16, name=f"kraw{j}") for j in range(NKT)]
            vext = raw.tile([128, NKT, HKV, 132], BF16, name="vext")
            # gate the gpsimd DMA queue behind the startup-critical weight/x
            # stream: everything below (ones, rope tables, masks) is needed
            # much later but would otherwise race for DMA bandwidth now
            gate = singles.tile([128, 1], BF16)
            nc.gpsimd.tensor_scalar_add(gate, xts[3][:, 3, 0:1], 0.0)
            nc.gpsimd.dma_start(out=vext[:, :, :, 128:129], in_=vld.ap())

            # per-key-tile exp scale: 1/rms(k) * 1/sqrt(dh), fp32 columns
            kinv = singles.tile([128, NKT, HKV], F32)
            # per-(qi,head) 1/rms(q)
            qinv = [singles.tile([128, HQ], F32, name=f"qinv{i}") for i in range(NQT)]

            # ---------------- QKV projection (k-outer, 4 psum groups) ------
            # feature columns: n=0..3 Q (heads 4n..4n+3), n=4 K, n=5 V
            def proj_col(n, wcs):
                itiles = list(range(4, 8)) if n < 4 else list(range(8))
                for half in range(len(itiles) // 4):
                    igrp = itiles[half * 4:half * 4 + 4]
                    pps = {i: pp_pool.tile([128, 512], F32, tag="pp", name=f"pp{n}_{i}")
                           for i in igrp}
                    for k in range(16):
                        for i in igrp:
                            nc.tensor.matmul(pps[i],
                                             xts[k // 4][:, k % 4, i * 128:(i + 1) * 128],
                                             wcs[k // 4][:, k % 4, :],
                                             start=(k == 0), stop=(k == 15))
                    for i in igrp:
                        if n < 4:
                            qi = i - 4
                            nc.scalar.copy(
                                out=qraw[qi][:, 4 * n:4 * n + 4, :].rearrange("p a b -> p (a b)"),
                                in_=pps[i][:])
                        elif n == 4:
                            nc.scalar.copy(
                                out=kraw[i][:, :, :].rearrange("p a b -> p (a b)"),
                                in_=pps[i][:])
                        else:
                            nc.scalar.copy(
                                out=vext[:, i, :, 0:128],
                                in_=pps[i].rearrange("p (g d) -> p g d", g=HKV))

            # ---------------- RoPE + RMS-norm (row layout) -----------------
            qt = singles.tile([128, HQ, NQT, 128], BF16)   # [dh, h, qi, tok]
            kt = singles.tile([128, HKV, NKT, 128], BF16)  # [dh, g, j, tok]
            # rope'd Q rows accumulate per qi (one block transpose per qi)
            roq = [singles.tile([128, HQ, DH], BF16, name=f"roq{i}") for i in range(NQT)]

            def rope(src, h, sfx, cs_src, ss_src, ro, rinv_out):
                # src: [128 tok, h, 128 dh] bf16 raw. Writes rope'd rows into
                # ro (unnormalized) and exp(-0.5*ln(ms+eps))[+lnscale] to
                # rinv_out ([128, h] f32). Norms commute with rotation.
                cs = tbl.tile([128, h, DH], BF16, tag=f"cs{sfx}", name=f"cs{sfx}")
                ss = tbl.tile([128, h, DH], BF16, tag=f"ss{sfx}", name=f"ss{sfx}")
                nc.gpsimd.dma_start(out=cs.rearrange("p a b -> p (a b)"), in_=cs_src)
                nc.gpsimd.dma_start(out=ss.rearrange("p a b -> p (a b)"), in_=ss_src)

                tmp = work.tile([128, h, DH], BF16, tag=f"tmp{sfx}")
                nc.vector.tensor_mul(tmp, src, src)
                ssum = work.tile([128, h], F32, tag=f"ssum{sfx}")
                nc.vector.reduce_sum(out=ssum, in_=tmp, axis=mybir.AxisListType.X)
                rstd = work.tile([128, h], F32, tag=f"rstd{sfx}")
                if sfx == "k":
                    nc.scalar.activation(rstd, ssum, Sqrt, bias=epsk_t[:], scale=1.0)
                else:
                    nc.scalar.activation(rstd, ssum, Sqrt, bias=eps_t[:], scale=1.0 / DH)
                nc.vector.reciprocal(rinv_out, rstd)

                nc.vector.tensor_mul(tmp[:, :, 0:64], src[:, :, 64:128], ss[:, :, 0:64])
                nc.vector.tensor_mul(tmp[:, :, 64:128], src[:, :, 0:64], ss[:, :, 64:128])
                nc.vector.tensor_mul(ro, src, cs)
                nc.vector.tensor_add(ro, ro, tmp)

            # Q columns first; their rope trails each column so everything is
            # transposed well before attention. K then V.
            for n in range(4):
                wcs = wq0 if n == 0 else [load_wq4(n, c4, f"w{n}_{c4}")
                                          for c4 in range(4)]
                proj_col(n, wcs)
                for qi in range(NQT):
                    ro_sl = roq[qi][:, 4 * n:4 * n + 4, :]
                    rope(qraw[qi][:, 4 * n:4 * n + 4, :], 4, "q",
                         csq_r[qi][:, n * 512:(n + 1) * 512],
                         ssq_r[qi][:, n * 512:(n + 1) * 512],
                         ro_sl, qinv[qi][:, 4 * n:4 * n + 4])
                    for hh in range(4):
                        nc.vector.tensor_scalar_mul(
                            ro_sl[:, hh, :], ro_sl[:, hh, :],
                            qinv[qi][:, 4 * n + hh:4 * n + hh + 1])
                    if n == 3:
                        nc.scalar.dma_start_transpose(
                            out=qt[:, :, qi, :],
                            in_=roq[qi].rearrange("p a b -> p (a b)"))

            wck = [load_wq4(4, c4, f"wk_{c4}") for c4 in range(4)]
            proj_col(4, wck)
            wcv = [load_wq4(5, c4, f"wv_{c4}") for c4 in range(4)]
            for j in range(NKT):
                rok = work.tile([128, HKV, DH], BF16, tag="rok", bufs=3)
                rope(kraw[j], HKV, "k", csk_r[j], ssk_r[j], rok, kinv[:, j, :])
                nc.scalar.dma_start_transpose(out=kt[:, :, j, :],
                                              in_=rok.rearrange("p a b -> p (a b)"))
            proj_col(5, wcv)

            # band-edge masks (single DMA)
            msk = singles.tile([128, NQT, 2, 128], BF16)
            nc.gpsimd.dma_start(out=msk.rearrange("p a b c -> p (a b c)"),
                                in_=mask.ap().rearrange("i p m -> p i m"))

            # out-proj weight columns, streamed through the same quarter pool
            wpc = []
            for n in range(4):
                wpc.append([wtiles.tile([128, 4, 512], BF16, tag="wc4", name=f"wp{n}_{c4}")
                            for c4 in range(4)])
                for c4 in range(4):
                    nc.sync.dma_start(
                        out=wpc[n][c4],
                        in_=wp_r[:, 4 * c4:4 * c4 + 4, n * 512:(n + 1) * 512])

            # ---------------- attention ----------------
            # yt per head: [128 dh, qi, 128 tok] bf16
            yt = [singles.tile([128, NQT, 128], BF16, name=f"yt{h}") for h in range(HQ)]
            for h in range(HQ):
                g = h // 4
                # scores per key tile jk, batched over the query tiles using it;
                # K's rms-norm and the 1/sqrt(dh) ride the exp scale (per key)
                pes = {}
                for jk in range(NKT):
                    qlo, qhi = max(0, jk - 4), min(NQT - 1, jk)
                    nq = qhi - qlo + 1
                    sc = psA.tile([128, 512], F32, tag="sc", name="sc")
                    nc.tensor.matmul(sc[:, 0:nq * 128], kt[:, g, jk, :],
                                     qt[:, h, qlo:qhi + 1, :], start=True, stop=True)
                    pe = pepool.tile([128, 512], BF16, tag="pe", name="pe")
                    nc.scalar.activation(pe[:, 0:nq * 128], sc[:, 0:nq * 128],
                                         Exp, scale=kinv[:, jk, g:g + 1])
                    # band-edge masks: diagonal block (qi == jk) and far block
                    # (qi == jk - 4)
                    if jk <= NQT - 1:
                        c0 = (jk - qlo) * 128
                        nc.vector.tensor_mul(pe[:, c0:c0 + 128], pe[:, c0:c0 + 128],
                                             msk[:, jk, 0, :])
                    if jk >= 4:
                        nc.vector.tensor_mul(pe[:, 0:128], pe[:, 0:128],
                                             msk[:, jk - 4, 1, :])
                    pes[jk] = (pe, qlo)
                ysb = work.tile([128, NQT, 128], BF16, tag="ysb", bufs=3)
                for qi in range(NQT):
                    yv = psB.tile([128, 256], F32, tag="yv")
                    for j in range(5):
                        jk = qi + j
                        pe, qlo = pes[jk]
                        c0 = (qi - qlo) * 128
                        nc.tensor.matmul(yv[:, 0:129], pe[:, c0:c0 + 128],
                                         vext[:, jk, g, 0:129],
                                         start=(j == 0), stop=(j == 4))
                    linv = work.tile([128, 1], F32, tag="linv", bufs=6)
                    nc.vector.reciprocal(linv, yv[:, 128:129])
                    nc.vector.tensor_scalar_mul(ysb[:, qi, :], yv[:, 0:128], linv)
                # sync queue: idle during attention, keeps ACT free for exp
                nc.sync.dma_start_transpose(out=yt[h],
                                            in_=ysb.rearrange("p a b -> p (a b)"))

            # ---------------- output projection ----------------
            for n in range(4):
                for i in range(NQT):
                    po = pp_pool.tile([128, 512], F32, tag="pp", name="po")
                    for h in range(HQ):
                        nc.tensor.matmul(po, yt[h][:, i, :], wpc[n][h // 4][:, h % 4, :],
                                         start=(h == 0), stop=(h == 15))
                    osb = work.tile([128, 512], F32, tag="osb", bufs=3)
                    nc.scalar.copy(out=osb, in_=po)
                    nc.sync.dma_start(
                        out=out.ap()[i * 128:(i + 1) * 128, n * 512:(n + 1) * 512],
                        in_=osb)
    nc.compile()
    return nc


def _host_prep(x, w_qkv, w_proj):
    """Per-core input maps (numpy, bf16)."""
    bf = ml_dtypes.bfloat16
    # de-interleave perm within each Q/K head: new j <- old sigma(j)
    sig = np.empty(DH, np.int64)
    sig[:64] = np.arange(64) * 2
    sig[64:] = np.arange(64) * 2 + 1
    wqp = w_qkv.copy()
    for h in range(HQ + HKV):          # Q heads then K heads share layout
        base = h * DH
        wqp[base:base + DH] = w_qkv[base + sig]
    wqT = np.ascontiguousarray(wqp.T).astype(bf)          # [C, 3072]
    wpT = np.ascontiguousarray(w_proj.T).astype(bf)       # [C, C]

    inv_freq = 1.0 / (10000.0 ** (np.arange(0, DH, 2, dtype=np.float64) / DH))

    def tables(pos):  # pos [n] -> cos/sin [n, DH] (rotate-half, sign-baked sin)
        f = pos[:, None].astype(np.float64) * inv_freq[None, :]
        cs = np.concatenate([np.cos(f), np.cos(f)], axis=1)
        ss = np.concatenate([-np.sin(f), np.sin(f)], axis=1)
        return cs.astype(np.float32), ss.astype(np.float32)

    in_maps = []
    for core in range(8):
        b, c = divmod(core, 4)
        own0 = c * CHUNK
        w0 = own0 - CHUNK                     # window start (may be negative)
        xw = np.zeros((WTOK, C), np.float32)
        lo = max(0, w0)
        xw[lo - w0:, :] = x[b, lo:own0 + CHUNK, :]
        xtw = np.ascontiguousarray(xw.T).astype(bf)       # [C, 1024]

        kpos = np.maximum(np.arange(w0, own0 + CHUNK), 0)
        cskv, sskv = tables(kpos)                          # [1024, 128]
        csq1, ssq1 = tables(np.arange(own0, own0 + CHUNK))  # [512, 128]

        # triangle masks (j=0, j=4) in P^T layout: [qi, key_local_p, (2, query)]
        mask = np.zeros((NQT, 128, 256), np.float32)
        for i in range(NQT):
            qp = own0 + i * 128 + np.arange(128)[:, None]
            kp = w0 + i * 128 + np.arange(KWIN)[None, :]
            m_qk = ((kp >= 0) & (kp <= qp) & (qp - kp < WIN)).astype(np.float32)
            mt = m_qk.T.reshape(5, 128, 128)      # [j, k_local, q]
            mask[i] = np.concatenate([mt[0], mt[4]], axis=1)
        # [128, jk, g]: valid flag per key tile row, replicated over kv heads
        vldv = np.repeat((np.arange(w0, own0 + CHUNK) >= 0)
                         .astype(np.float32).reshape(NKT, 128).T[:, :, None],
                         HKV, axis=2).reshape(128, NKT * HKV)

        in_maps.append({
            "xt": xtw,
            "wq": wqT,
            "wp": wpT,
            "csq": np.tile(csq1, (1, HQ)).astype(bf),
            "ssq": np.tile(ssq1, (1, HQ)).astype(bf),
            "csk": np.tile(cskv, (1, HKV)).astype(bf),
            "ssk": np.tile(sskv, (1, HKV)).astype(bf),
            "mask": mask.astype(bf),
            "vld": vldv.astype(bf),
        })
    return in_maps


def kernel(x, w_qkv, w_proj, _trace=False):
    if _NC_CACHE[0] is None:
        _NC_CACHE[0] = _build_nc()
    nc = _NC_CACHE[0]
    in_maps = _host_prep(np.asarray(x, np.float32), np.asarray(w_qkv, np.float32),
                         np.asarray(w_proj, np.float32))
    res = run_bass_kernel_spmd(nc, in_maps, core_ids=list(range(8)), trace=_trace)
    outs = [res.results[i]["out"] for i in range(8)]
    full = np.empty((B, T, C), np.float32)
    for core in range(8):
        b, c = divmod(core, 4)
        full[b, c * CHUNK:(c + 1) * CHUNK] = outs[core]
    if _trace:
        kernel.last_exec_time_ns = res.exec_time_ns
        kernel.last_results = res
    return full
